# revision 16
# baseline (speedup 1.0000x reference)
"""DBRX MoE block (router + top-2 expert GLU FFN) on 8 Trainium2 cores.

Strategy:
- Host: router (softmax + top-2, replicates jax.lax.top_k semantics),
  token gather per expert, weight layout permutation.
- Device (expert-parallel, core e owns expert e): fused GLU FFN
  h1 = x @ w1.T ; gate = x @ v1.T ; act = silu(h1)*gate ; out = act @ w2.T
  for the tokens routed to that expert (capacity-padded to the max count).
  Matmuls run in float32r (full-rate fp32 streaming, ~1.5e-4 rms err).
- Host: weighted scatter-add combine of per-expert outputs.
"""

import os

import numpy as np

import concourse.bass as bass
import concourse.mybir as mybir
import concourse.tile as tile
from concourse import bacc
from concourse.bass_utils import run_bass_kernel_spmd

E, TOP_K, H, F = 8, 2, 2048, 5632
KT = H // 128   # 16 contraction subtiles for MM1
FT = F // 128   # 44 f tiles
FH = FT // 2    # f tiles per half (act buffer holds one half)
HC = 4          # output H chunks of 512
MM_DT = mybir.dt.float32r

_prog_cache: dict[int, object] = {}
LAST_EXEC_NS = None


def _build_program(Cpad: int):
    """Per-core fused MoE FFN for Cpad capacity-padded tokens."""
    C1 = (Cpad + 1) // 2
    chunks = [(0, C1), (C1, Cpad - C1)]
    TS = -(-Cpad // 128)  # t subtiles for MM2 stationary

    nc = bacc.Bacc(None, target_bir_lowering=False)
    # Layouts are host-pre-tiled so every DMA is one contiguous block per
    # partition (large bursts, few descriptors).
    xgT_d = nc.dram_tensor("xgT", [128, KT, Cpad], MM_DT, kind="ExternalInput")
    w1t_d = nc.dram_tensor("w1t", [FT, 128, KT, 128], MM_DT, kind="ExternalInput")
    v1t_d = nc.dram_tensor("v1t", [FT, 128, KT, 128], MM_DT, kind="ExternalInput")
    w2t_d = nc.dram_tensor("w2t", [FT, HC, 128, 512], MM_DT, kind="ExternalInput")
    out_d = nc.dram_tensor("out", [Cpad, H], mybir.dt.float32, kind="ExternalOutput")

    with tile.TileContext(nc) as tc:
        with (
            tc.tile_pool(name="xpool", bufs=1) as xpool,
            tc.tile_pool(name="apool", bufs=1) as apool,
            tc.tile_pool(name="opool", bufs=1) as opool,
            tc.tile_pool(name="wpool", bufs=3) as wpool,
            tc.tile_pool(name="w2pool", bufs=4) as w2pool,
            tc.tile_pool(name="ps1", bufs=1, space="PSUM") as ps1,
            tc.tile_pool(name="ps2", bufs=1, space="PSUM") as ps2,
        ):
            xg = xpool.tile([128, KT, Cpad], MM_DT, tag="xg")
            nc.sync.dma_start(xg[:, 0, :], xgT_d[:, 0, :])
            out_sb = opool.tile([128, TS, H], mybir.dt.float32, tag="osb")

            for half in range(2):
                f_lo = half * FH
                act = apool.tile([128, FH, Cpad], MM_DT, tag="act")
                # ---- MM1: h1/gate + silu*gate into act ----
                for fi in range(FH):
                    ft = f_lo + fi
                    w1tile = wpool.tile([128, KT, 128], MM_DT, tag="w1")
                    nc.sync.dma_start(w1tile[:], w1t_d[ft])
                    v1tile = wpool.tile([128, KT, 128], MM_DT, tag="v1")
                    nc.sync.dma_start(v1tile[:], v1t_d[ft])
                    if half == 0 and fi == 0:
                        # remaining xg k-slices: queued after the first weight
                        # tiles so the first matmul isn't stuck behind them,
                        # but before any matmul that reads them (deps follow
                        # emission order)
                        for k in range(1, KT):
                            nc.sync.dma_start(xg[:, k, :], xgT_d[:, k, :])
                    for (c0, cn) in chunks:
                        # h1 double-buffered so next chunk's matmuls can start
                        # while silu/mul drain the previous one.
                        ph = ps1.tile([128, cn], mybir.dt.float32, tag="h1", bufs=2)
                        pg = ps1.tile([128, cn], mybir.dt.float32, tag="gate")
                        for k in range(KT):
                            nc.tensor.matmul(
                                ph[:], w1tile[:, k, :], xg[:, k, c0:c0 + cn],
                                start=(k == 0), stop=(k == KT - 1),
                            )
                        for k in range(KT):
                            nc.tensor.matmul(
                                pg[:], v1tile[:, k, :], xg[:, k, c0:c0 + cn],
                                start=(k == 0), stop=(k == KT - 1),
                            )
                        a_slice = act[:, fi, c0:c0 + cn]
                        nc.scalar.activation(a_slice, ph[:], mybir.ActivationFunctionType.Silu)
                        nc.vector.tensor_mul(a_slice, a_slice, pg[:])

                # ---- MM2: out[t, h] += act.T @ w2t, accumulated over f ----
                for hc in range(HC):
                    pos = [
                        ps2.tile([128, 512], mybir.dt.float32, tag=f"po{ts}", name=f"po{ts}_{half}_{hc}")
                        for ts in range(TS)
                    ]
                    for fi in range(FH):
                        ft = f_lo + fi
                        w2tile = w2pool.tile([128, 512], MM_DT, tag="w2")
                        nc.sync.dma_start(w2tile[:], w2t_d[ft, hc])
                        for ts in range(TS):
                            pn = min(128, Cpad - ts * 128)
                            nc.tensor.matmul(
                                pos[ts][:pn, :],
                                act[:, fi, ts * 128:ts * 128 + pn],
                                w2tile[:],
                                start=(fi == 0), stop=(fi == FH - 1),
                            )
                    for ts in range(TS):
                        pn = min(128, Cpad - ts * 128)
                        dst = out_sb[:pn, ts, hc * 512:(hc + 1) * 512]
                        if half == 0:
                            nc.scalar.copy(dst, pos[ts][:pn, :])
                        else:
                            nc.vector.tensor_add(dst, dst, pos[ts][:pn, :])
                            nc.sync.dma_start(
                                out_d[ts * 128:ts * 128 + pn, hc * 512:(hc + 1) * 512], dst
                            )

    nc.finalize()
    return nc


def _router(xt: np.ndarray, Wr: np.ndarray):
    """Replicates reference router in fp32 numpy (matches jax.lax.top_k)."""
    logits = xt @ Wr.T
    m = logits.max(axis=-1, keepdims=True)
    ex = np.exp(logits - m, dtype=np.float32)
    weights = ex / ex.sum(axis=-1, keepdims=True)
    order = np.argsort(-weights, axis=-1, kind="stable")[:, :TOP_K]
    top_w = np.take_along_axis(weights, order, axis=-1)
    top_w = top_w / np.abs(top_w).sum(axis=-1, keepdims=True)
    return weights.astype(np.float32), order, top_w.astype(np.float32)


def kernel(x, Wr, w1, v1, w2):
    x = np.asarray(x, dtype=np.float32)
    Wr = np.asarray(Wr, dtype=np.float32)
    w1 = np.asarray(w1, dtype=np.float32)
    v1 = np.asarray(v1, dtype=np.float32)
    w2 = np.asarray(w2, dtype=np.float32)
    b, s, h = x.shape
    T = b * s
    xt = x.reshape(T, h)

    weights, order, top_w = _router(xt, Wr)

    # token lists per expert
    idxs, wts = [], []
    for e in range(E):
        sel0 = np.nonzero(order[:, 0] == e)[0]
        sel1 = np.nonzero(order[:, 1] == e)[0]
        idxs.append(np.concatenate([sel0, sel1]))
        wts.append(np.concatenate([top_w[sel0, 0], top_w[sel1, 1]]))
    counts = np.array([len(i) for i in idxs])
    # multiple of 4: both MM1 moving chunks (Cpad/2) must be even for f32r
    Cpad = int(max(4, ((counts.max() + 3) // 4) * 4))

    nc = _prog_cache.get(Cpad)
    if nc is None:
        nc = _build_program(Cpad)
        _prog_cache[Cpad] = nc

    in_maps = []
    for e in range(E):
        xg = np.zeros((Cpad, H), dtype=np.float32)
        xg[:counts[e]] = xt[idxs[e]]
        xgT = np.ascontiguousarray(xg.T.reshape(KT, 128, Cpad).transpose(1, 0, 2))
        # [ft, p, k, fq] = w1[e][ft*128+fq, k*128+p]
        w1t = np.ascontiguousarray(w1[e].reshape(FT, 128, KT, 128).transpose(0, 3, 2, 1))
        v1t = np.ascontiguousarray(v1[e].reshape(FT, 128, KT, 128).transpose(0, 3, 2, 1))
        # [ft, hc, p, n] = w2[e][hc*512+n, ft*128+p]
        w2t = np.ascontiguousarray(w2[e].reshape(HC, 512, FT, 128).transpose(2, 0, 3, 1))
        in_maps.append({"xgT": xgT, "w1t": w1t, "v1t": v1t, "w2t": w2t})

    trace = os.environ.get("BASS_MOE_TRACE", "0") == "1"
    res = run_bass_kernel_spmd(nc, in_maps, core_ids=list(range(E)), trace=trace)
    global LAST_EXEC_NS
    LAST_EXEC_NS = res.exec_time_ns

    out = np.zeros((T, H), dtype=np.float32)
    for e in range(E):
        n = counts[e]
        if n:
            out[idxs[e]] += wts[e][:, None] * res.results[e]["out"][:n]

    return out.reshape(b, s, h), weights


# revision 17
# speedup vs baseline: 1.1735x; 1.1735x over previous
"""DBRX MoE block (router + top-2 expert GLU FFN) on 8 Trainium2 cores.

Strategy:
- Host: router (softmax + top-2, replicates jax.lax.top_k semantics),
  token gather per expert, weight layout permutation.
- Device (expert-parallel, core e owns expert e): fused GLU FFN
  h1 = x @ w1.T ; gate = x @ v1.T ; act = silu(h1)*gate ; out = act @ w2.T
  for the tokens routed to that expert (capacity-padded to the max count).
  Matmuls run in float32r (full-rate fp32 streaming, ~1.5e-4 rms err).
- Host: weighted scatter-add combine of per-expert outputs.
"""

import os

import numpy as np

import concourse.bass as bass
import concourse.mybir as mybir
import concourse.tile as tile
from concourse import bacc
from concourse.bass_utils import run_bass_kernel_spmd

E, TOP_K, H, F = 8, 2, 2048, 5632
KT = H // 128   # 16 contraction subtiles for MM1
FT = F // 128   # 44 f tiles
FH = FT // 2    # f tiles per half (act buffer holds one half)
HC = 4          # output H chunks of 512
MM_DT = mybir.dt.float32r

_prog_cache: dict[int, object] = {}
LAST_EXEC_NS = None


def _build_program(Cpad: int):
    """Per-core fused MoE FFN for Cpad capacity-padded tokens."""
    C1 = (Cpad + 1) // 2
    chunks = [(0, C1), (C1, Cpad - C1)]
    TS = -(-Cpad // 128)  # t subtiles for MM2 stationary

    nc = bacc.Bacc(None, target_bir_lowering=False)
    # Layouts are host-pre-tiled so every DMA is one contiguous block per
    # partition (large bursts, few descriptors).
    xgT_d = nc.dram_tensor("xgT", [128, KT, Cpad], MM_DT, kind="ExternalInput")
    w1t_d = nc.dram_tensor("w1t", [FT, 128, KT, 128], MM_DT, kind="ExternalInput")
    v1t_d = nc.dram_tensor("v1t", [FT, 128, KT, 128], MM_DT, kind="ExternalInput")
    w2t_d = nc.dram_tensor("w2t", [FT, HC, 128, 512], MM_DT, kind="ExternalInput")
    out_d = nc.dram_tensor("out", [Cpad, H], mybir.dt.float32, kind="ExternalOutput")

    with tile.TileContext(nc) as tc:
        with (
            tc.tile_pool(name="xpool", bufs=1) as xpool,
            tc.tile_pool(name="apool", bufs=1) as apool,
            tc.tile_pool(name="opool", bufs=1) as opool,
            tc.tile_pool(name="wpool", bufs=3) as wpool,
            tc.tile_pool(name="w2pool", bufs=4) as w2pool,
            tc.tile_pool(name="ps1", bufs=1, space="PSUM") as ps1,
            tc.tile_pool(name="ps2", bufs=1, space="PSUM") as ps2,
        ):
            xg = xpool.tile([128, KT, Cpad], MM_DT, tag="xg")
            nc.sync.dma_start(xg[:, 0, :], xgT_d[:, 0, :])
            out_sb = opool.tile([128, TS, H], mybir.dt.float32, tag="osb")

            for half in range(2):
                f_lo = half * FH
                act = apool.tile([128, FH, Cpad], MM_DT, tag="act")
                # ---- MM1: h1/gate + silu*gate into act ----
                for fi in range(FH):
                    ft = f_lo + fi
                    w1tile = wpool.tile([128, KT, 128], MM_DT, tag="w1")
                    nc.sync.dma_start(w1tile[:], w1t_d[ft])
                    v1tile = wpool.tile([128, KT, 128], MM_DT, tag="v1")
                    nc.sync.dma_start(v1tile[:], v1t_d[ft])
                    if half == 0 and fi == 0:
                        # remaining xg k-slices: queued after the first weight
                        # tiles so the first matmul isn't stuck behind them,
                        # but before any matmul that reads them (deps follow
                        # emission order)
                        for k in range(1, KT):
                            nc.sync.dma_start(xg[:, k, :], xgT_d[:, k, :])
                    for (c0, cn) in chunks:
                        # h1 double-buffered so next chunk's matmuls can start
                        # while silu/mul drain the previous one.
                        ph = ps1.tile([128, cn], mybir.dt.float32, tag="h1", bufs=2)
                        pg = ps1.tile([128, cn], mybir.dt.float32, tag="gate")
                        for k in range(KT):
                            nc.tensor.matmul(
                                ph[:], w1tile[:, k, :], xg[:, k, c0:c0 + cn],
                                start=(k == 0), stop=(k == KT - 1),
                            )
                        for k in range(KT):
                            nc.tensor.matmul(
                                pg[:], v1tile[:, k, :], xg[:, k, c0:c0 + cn],
                                start=(k == 0), stop=(k == KT - 1),
                            )
                        a_slice = act[:, fi, c0:c0 + cn]
                        nc.scalar.activation(a_slice, ph[:], mybir.ActivationFunctionType.Silu)
                        nc.vector.tensor_mul(a_slice, a_slice, pg[:])

                # ---- MM2: out[t, h] += act.T @ w2t, accumulated over f ----
                for hc in range(HC):
                    pos = [
                        ps2.tile([128, 512], mybir.dt.float32, tag=f"po{ts}", name=f"po{ts}_{half}_{hc}")
                        for ts in range(TS)
                    ]
                    for fi in range(FH):
                        ft = f_lo + fi
                        w2tile = w2pool.tile([128, 512], MM_DT, tag="w2")
                        nc.sync.dma_start(w2tile[:], w2t_d[ft, hc])
                        for ts in range(TS):
                            pn = min(128, Cpad - ts * 128)
                            nc.tensor.matmul(
                                pos[ts][:pn, :],
                                act[:, fi, ts * 128:ts * 128 + pn],
                                w2tile[:],
                                start=(fi == 0), stop=(fi == FH - 1),
                            )
                    for ts in range(TS):
                        pn = min(128, Cpad - ts * 128)
                        dst = out_sb[:pn, ts, hc * 512:(hc + 1) * 512]
                        if half == 0:
                            nc.scalar.copy(dst, pos[ts][:pn, :])
                        else:
                            nc.vector.tensor_add(dst, dst, pos[ts][:pn, :])
                            # scalar HWDGE ring: keeps HBM-write completion
                            # latency out of the sync ring feeding weight loads
                            nc.scalar.dma_start(
                                out_d[ts * 128:ts * 128 + pn, hc * 512:(hc + 1) * 512], dst
                            )

    nc.finalize()
    return nc


def _router(xt: np.ndarray, Wr: np.ndarray):
    """Replicates reference router in fp32 numpy (matches jax.lax.top_k)."""
    logits = xt @ Wr.T
    m = logits.max(axis=-1, keepdims=True)
    ex = np.exp(logits - m, dtype=np.float32)
    weights = ex / ex.sum(axis=-1, keepdims=True)
    order = np.argsort(-weights, axis=-1, kind="stable")[:, :TOP_K]
    top_w = np.take_along_axis(weights, order, axis=-1)
    top_w = top_w / np.abs(top_w).sum(axis=-1, keepdims=True)
    return weights.astype(np.float32), order, top_w.astype(np.float32)


def kernel(x, Wr, w1, v1, w2):
    x = np.asarray(x, dtype=np.float32)
    Wr = np.asarray(Wr, dtype=np.float32)
    w1 = np.asarray(w1, dtype=np.float32)
    v1 = np.asarray(v1, dtype=np.float32)
    w2 = np.asarray(w2, dtype=np.float32)
    b, s, h = x.shape
    T = b * s
    xt = x.reshape(T, h)

    weights, order, top_w = _router(xt, Wr)

    # token lists per expert
    idxs, wts = [], []
    for e in range(E):
        sel0 = np.nonzero(order[:, 0] == e)[0]
        sel1 = np.nonzero(order[:, 1] == e)[0]
        idxs.append(np.concatenate([sel0, sel1]))
        wts.append(np.concatenate([top_w[sel0, 0], top_w[sel1, 1]]))
    counts = np.array([len(i) for i in idxs])
    # multiple of 4: both MM1 moving chunks (Cpad/2) must be even for f32r
    Cpad = int(max(4, ((counts.max() + 3) // 4) * 4))

    nc = _prog_cache.get(Cpad)
    if nc is None:
        nc = _build_program(Cpad)
        _prog_cache[Cpad] = nc

    in_maps = []
    for e in range(E):
        xg = np.zeros((Cpad, H), dtype=np.float32)
        xg[:counts[e]] = xt[idxs[e]]
        xgT = np.ascontiguousarray(xg.T.reshape(KT, 128, Cpad).transpose(1, 0, 2))
        # [ft, p, k, fq] = w1[e][ft*128+fq, k*128+p]
        w1t = np.ascontiguousarray(w1[e].reshape(FT, 128, KT, 128).transpose(0, 3, 2, 1))
        v1t = np.ascontiguousarray(v1[e].reshape(FT, 128, KT, 128).transpose(0, 3, 2, 1))
        # [ft, hc, p, n] = w2[e][hc*512+n, ft*128+p]
        w2t = np.ascontiguousarray(w2[e].reshape(HC, 512, FT, 128).transpose(2, 0, 3, 1))
        in_maps.append({"xgT": xgT, "w1t": w1t, "v1t": v1t, "w2t": w2t})

    trace = os.environ.get("BASS_MOE_TRACE", "0") == "1"
    res = run_bass_kernel_spmd(nc, in_maps, core_ids=list(range(E)), trace=trace)
    global LAST_EXEC_NS
    LAST_EXEC_NS = res.exec_time_ns

    out = np.zeros((T, H), dtype=np.float32)
    for e in range(E):
        n = counts[e]
        if n:
            out[idxs[e]] += wts[e][:, None] * res.results[e]["out"][:n]

    return out.reshape(b, s, h), weights
